# revision 1
# baseline (speedup 1.0000x reference)
"""Trainium2 Bass kernel for nn_LutLayer (B=512, depth=4096, SIX=6).

Math: per element with x = inputs[b, d, :] (6 values),
    out = sum_{i=0}^{63} w_i * prod_j q_{j, bit_j(i)},
    q_{j,1} = (1-x_j)+eps, q_{j,0} = x_j+eps,  w_i = g(count0(i)),
    g(c) = logit(clamp(c/6)).

Since w depends only on popcount, out = sum_c g_c * S_c where S_c are the
Poisson-binomial coefficients of prod_j (v_j + u_j t).  The sequence g_c is
annihilated by a palindromic quartic with a double root at t=1 and the
reciprocal pair {tau, 1/tau}, giving the exact closed form

    out = C0 + C1 * sum_j y_j + S3 * [prod_j (y_j + D0) - prod_j (y_j - D0)]

with y_j = 2 x_j - 1 and D0 = (1+2eps)(1+tau)/(tau-1).  Only two 6-factor
products and one 6-term sum per element remain; |S3|^(1/6) is folded into
the affine factors so all intermediates stay O(1) in fp32.

Sharding: data-parallel over batch, 64 batches per core on 8 cores.
"""

import sys
from contextlib import ExitStack

import numpy as np

if "/opt/trn_rl_repo" not in sys.path:
    sys.path.insert(0, "/opt/trn_rl_repo")

import concourse.bass as bass
import concourse.tile as tile
from concourse import mybir
from concourse.bass_utils import run_bass_kernel_spmd

N_CORES = 8
B, DEPTH, SIX = 512, 4096, 6
PER_CORE_B = B // N_CORES            # 64
N_ELEM = PER_CORE_B * DEPTH          # 262144 elements per core
P = 128                              # SBUF partitions
FD_TOT = N_ELEM // P                 # 2048 elements per partition
CHUNK = 512                          # elements per partition per chunk
N_CHUNKS = FD_TOT // CHUNK           # 4

# exact decomposition constants (fp64, derived offline; see module docstring)
D0 = 1.244957288028531
S3 = 0.020370985329978712
C1 = 0.33123508857995426
C0 = 1.0089040713978648e-11
W = S3 ** (1.0 / 6.0)                # folded branch weight, 0.52259911...

SCALE_F = float(2.0 * W)             # scale for both product branches
BIAS_P = float(W * (D0 - 1.0))       # bias for (y + D0) branch
BIAS_N = float(W * (-D0 - 1.0))      # bias for (y - D0) branch
LIN_SCALE = float(2.0 * C1)          # applied to sum_j x_j
LIN_BIAS = float(C0 - 6.0 * C1)      # C0 + C1 * (-6)
# linear branch is computed from sum_j F3_j = SCALE_F*sum_j x_j + 6*BIAS_P
# (keeps the input tile's readers on a single engine for sem-wait limits)
LIN_SCALE2 = float(LIN_SCALE / SCALE_F)
LIN_BIAS2 = float(LIN_BIAS - 6.0 * BIAS_P * LIN_SCALE / SCALE_F)

F32 = mybir.dt.float32

# walrus codegen caps sync-wait commands per instruction (empirically: 1 for
# DMACopy and Pool/GPSIMD ops, 2 for ACT/DVE compute).  Tile's sem assignment
# can exceed that, so excess waits are split onto a standalone EventSemaphore
# on the same engine queue (program order makes that equivalent; the final
# all-engine barrier already uses 15-wait EventSemaphores, so they're legal).
_SPLIT_SKIP = {"InstEventSemaphore", "InstUnconditionalBranch",
               "InstCall", "InstRegisterMove"}


def _wait_budget(inst):
    # Empirically every compute/DMA instruction struct accepts only ONE
    # sync-wait command (EventSemaphore accepts two).
    return 1


def _split_sync_waits(nc):
    for f in nc.m.functions:
        for b in f.blocks:
            new_insts = []
            for inst in b.instructions:
                si = inst.sync_info
                waits = list(si.on_wait) if si and si.on_wait else []
                budget = _wait_budget(inst)
                if type(inst).__name__ not in _SPLIT_SKIP and len(waits) > budget:
                    excess, keep = waits[:-budget], waits[-budget:]
                    for i in range(0, len(excess), 2):  # EventSemaphore: <=2 waits
                        ev = mybir.InstEventSemaphore(
                            name=f"{inst.name}-ws{i}",
                            opcode="EventSemaphore",
                            engine=inst.engine,
                            ins=[],
                            outs=[],
                            sync_info=mybir.SyncInfo(on_wait=excess[i:i + 2],
                                                     on_update=[]),
                            bass_nofuse=True,
                        )
                        new_insts.append(ev)
                    inst.sync_info = mybir.SyncInfo(on_wait=keep,
                                                    on_update=si.on_update)
                new_insts.append(inst)
            b.instructions = new_insts


def _build_bass(chunk=CHUNK, fp_bufs=2, o1_act=False, o2_pool=False, o3_pool=False,
                chunks=None, v4p4_pool=False, merged=False, accum=False):
    if chunks is None:
        chunks = [chunk] * (FD_TOT // chunk)
    assert sum(chunks) == FD_TOT, chunks
    n_chunks = len(chunks)
    nc = bass.Bass()
    x_in = nc.declare_dram_parameter("x", [P, FD_TOT * SIX], F32, isOutput=False)
    y_out = nc.declare_dram_parameter("out", [P, FD_TOT], F32, isOutput=True)

    with tile.TileContext(nc) as tc, ExitStack() as ctx:
        # Sync-wait budgets (walrus codegen): DMACopy and Pool(GPSIMD)
        # instructions tolerate only ONE wait command; ACT handles >=3.
        # So every tile written by DMA or read/written by GPSIMD gets one
        # buffer per chunk (no WAR waits at all), DVE-internal tiles are
        # bufs=1 (same-engine deps need no semaphores), and the remaining
        # multi-wait pressure (slot reuse of F3/F4) lands on ACT.
        xp = ctx.enter_context(tc.tile_pool(name="x", bufs=1))
        fp = ctx.enter_context(tc.tile_pool(name="fct", bufs=fp_bufs))
        tp = ctx.enter_context(tc.tile_pool(name="lvl1", bufs=1))
        vp = ctx.enter_context(tc.tile_pool(name="lvl23", bufs=1))
        s1p = ctx.enter_context(tc.tile_pool(name="sum1", bufs=1))
        slp = ctx.enter_context(tc.tile_pool(name="sum23", bufs=1))
        op = ctx.enter_context(tc.tile_pool(name="out", bufs=1))
        off = 0
        for t in range(n_chunks):
            chunk = chunks[t]
            X = xp.tile([P, chunk * SIX], F32, tag=f"x{t}")
            nc.sync.dma_start(X[:], x_in[:, off * SIX:(off + chunk) * SIX])
            Xv = X[:].rearrange("p (f s) -> p f s", s=SIX)

            if merged:
                # both product branches in one double-width pipeline
                FF = fp.tile([P, 2 * chunk * SIX], F32, tag="ff")
                nc.scalar.activation(FF[:, 0:chunk * SIX], X[:],
                                     mybir.ActivationFunctionType.Copy,
                                     bias=BIAS_P, scale=SCALE_F)
                nc.scalar.activation(FF[:, chunk * SIX:], X[:],
                                     mybir.ActivationFunctionType.Copy,
                                     bias=BIAS_N, scale=SCALE_F)
                FFv = FF[:].rearrange("p (b c s) -> p b c s", b=2, s=SIX)
                TT = tp.tile([P, 2 * chunk * 3], F32, tag="tt")
                TTw = TT[:].rearrange("p (b k c) -> p b c k", b=2, k=3)
                nc.vector.tensor_tensor(TTw, FFv[:, :, :, 0:3], FFv[:, :, :, 3:6],
                                        mybir.AluOpType.mult)
                TTv = TT[:].rearrange("p (b k c) -> p b k c", b=2, k=3)
                VV = vp.tile([P, 2 * chunk], F32, tag="vv")
                VVv = VV[:].rearrange("p (b c) -> p b c", b=2)
                nc.vector.tensor_tensor(VVv, TTv[:, :, 0, :], TTv[:, :, 1, :],
                                        mybir.AluOpType.mult)
                PP2 = vp.tile([P, 2 * chunk], F32, tag="pp2")
                PP2v = PP2[:].rearrange("p (b c) -> p b c", b=2)
                nc.vector.tensor_tensor(PP2v, VVv, TTv[:, :, 2, :],
                                        mybir.AluOpType.mult)
                PP = vp.tile([P, chunk], F32, tag="ppd")
                nc.vector.tensor_tensor(PP[:], PP2[:, 0:chunk], PP2[:, chunk:],
                                        mybir.AluOpType.subtract)
                # linear branch on gpsimd
                S1 = s1p.tile([P, chunk * 3], F32, tag=f"s1_{t}")
                S1w = S1[:].rearrange("p (s f) -> p f s", s=3)
                nc.gpsimd.tensor_tensor(S1w, Xv[:, :, 0:3], Xv[:, :, 3:6],
                                        mybir.AluOpType.add)
                S2 = slp.tile([P, chunk], F32, tag=f"s2_{t}")
                nc.gpsimd.tensor_tensor(S2[:], S1[:, 0:chunk], S1[:, chunk:2 * chunk],
                                        mybir.AluOpType.add)
                L = slp.tile([P, chunk], F32, tag=f"lsum{t}")
                nc.gpsimd.tensor_tensor(L[:], S2[:], S1[:, 2 * chunk:3 * chunk],
                                        mybir.AluOpType.add)
                O1 = op.tile([P, chunk], F32, tag=f"o1_{t}")
                if o1_act:
                    nc.scalar.activation(O1[:], L[:],
                                         mybir.ActivationFunctionType.Copy,
                                         bias=LIN_BIAS, scale=LIN_SCALE)
                else:
                    nc.vector.tensor_scalar(O1[:], L[:], LIN_SCALE, LIN_BIAS,
                                            mybir.AluOpType.mult,
                                            mybir.AluOpType.add)
                O3 = op.tile([P, chunk], F32, tag=f"o3_{t}")
                nc.vector.tensor_tensor(O3[:], O1[:], PP[:], mybir.AluOpType.add)
                nc.sync.dma_start(y_out[:, off:off + chunk], O3[:])
                off += chunk
                continue

            # product branches: factors w*(y +- D0) = SCALE_F*x + bias
            F3 = fp.tile([P, chunk * SIX], F32, tag="f3")
            nc.scalar.activation(F3[:], X[:], mybir.ActivationFunctionType.Copy,
                                 bias=BIAS_P, scale=SCALE_F)
            F4 = fp.tile([P, chunk * SIX], F32, tag="f4")
            nc.scalar.activation(F4[:], X[:], mybir.ActivationFunctionType.Copy,
                                 bias=BIAS_N, scale=SCALE_F)

            # level-1 pair products, written block-major: T[:, k*chunk+f]
            T3 = tp.tile([P, chunk * 3], F32, tag="t3")
            T3w = T3[:].rearrange("p (s f) -> p f s", s=3)
            F3v = F3[:].rearrange("p (f s) -> p f s", s=SIX)
            nc.vector.tensor_tensor(T3w, F3v[:, :, 0:3], F3v[:, :, 3:6],
                                    mybir.AluOpType.mult)
            T4 = tp.tile([P, chunk * 3], F32, tag="t4")
            T4w = T4[:].rearrange("p (s f) -> p f s", s=3)
            F4v = F4[:].rearrange("p (f s) -> p f s", s=SIX)
            nc.vector.tensor_tensor(T4w, F4v[:, :, 0:3], F4v[:, :, 3:6],
                                    mybir.AluOpType.mult)

            # levels 2-3 (contiguous block slices)
            V3 = vp.tile([P, chunk], F32, tag="v3")
            nc.vector.tensor_tensor(V3[:], T3[:, 0:chunk], T3[:, chunk:2 * chunk],
                                    mybir.AluOpType.mult)
            P3 = vp.tile([P, chunk], F32, tag="p3")
            nc.vector.tensor_tensor(P3[:], V3[:], T3[:, 2 * chunk:3 * chunk],
                                    mybir.AluOpType.mult)
            V4 = vp.tile([P, chunk], F32, tag=f"v4_{t}" if v4p4_pool else "v4")
            (nc.gpsimd if v4p4_pool else nc.vector).tensor_tensor(
                V4[:], T4[:, 0:chunk], T4[:, chunk:2 * chunk],
                                    mybir.AluOpType.mult)
            P4 = vp.tile([P, chunk], F32, tag=f"p4_{t}" if v4p4_pool else "p4")
            (nc.gpsimd if v4p4_pool else nc.vector).tensor_tensor(
                P4[:], V4[:], T4[:, 2 * chunk:3 * chunk],
                                    mybir.AluOpType.mult)

            # linear branch on gpsimd: L = sum_j x_j (tree), reading X directly
            # (X never carries WAR waits, and it keeps F3's readers DVE-only
            # so the ACT affines stay within their sync-wait budget)
            S1 = s1p.tile([P, chunk * 3], F32, tag=f"s1_{t}")
            S1w = S1[:].rearrange("p (s f) -> p f s", s=3)
            nc.gpsimd.tensor_tensor(S1w, Xv[:, :, 0:3], Xv[:, :, 3:6],
                                    mybir.AluOpType.add)
            S2 = slp.tile([P, chunk], F32, tag=f"s2_{t}")
            nc.gpsimd.tensor_tensor(S2[:], S1[:, 0:chunk], S1[:, chunk:2 * chunk],
                                    mybir.AluOpType.add)
            L = slp.tile([P, chunk], F32, tag=f"lsum{t}")
            nc.gpsimd.tensor_tensor(L[:], S2[:], S1[:, 2 * chunk:3 * chunk],
                                    mybir.AluOpType.add)

            if accum:
                # PP = P3 - P4 on DVE; O1 (linear part) written by ACT and
                # DMA'd as the base; PP accumulated into DRAM by SWDGE CCE.
                PP = vp.tile([P, chunk], F32, tag="ppd")
                nc.vector.tensor_tensor(PP[:], P3[:], P4[:],
                                        mybir.AluOpType.subtract)
                O1 = op.tile([P, chunk], F32, tag=f"o1_{t}")
                nc.scalar.activation(O1[:], L[:],
                                     mybir.ActivationFunctionType.Copy,
                                     bias=LIN_BIAS, scale=LIN_SCALE)
                nc.sync.dma_start(y_out[:, off:off + chunk], O1[:])
                nc.gpsimd.dma_start(y_out[:, off:off + chunk], PP[:],
                                    accum_op=mybir.AluOpType.add)
                off += chunk
                continue

            # combine: out = (LIN_SCALE*L + LIN_BIAS) + P3 - P4
            O1 = op.tile([P, chunk], F32, tag=f"o1_{t}")
            if o1_act:
                nc.scalar.activation(O1[:], L[:],
                                     mybir.ActivationFunctionType.Copy,
                                     bias=LIN_BIAS, scale=LIN_SCALE)
            else:
                nc.vector.tensor_scalar(O1[:], L[:], LIN_SCALE, LIN_BIAS,
                                        mybir.AluOpType.mult, mybir.AluOpType.add)
            O2 = op.tile([P, chunk], F32, tag=f"o2_{t}")
            (nc.gpsimd if o2_pool else nc.vector).tensor_tensor(
                O2[:], P3[:], P4[:], mybir.AluOpType.subtract)
            O3 = op.tile([P, chunk], F32, tag=f"o3_{t}")
            (nc.gpsimd if o3_pool else nc.vector).tensor_tensor(
                O3[:], O1[:], O2[:], mybir.AluOpType.add)

            nc.sync.dma_start(y_out[:, off:off + chunk], O3[:])
            off += chunk

    _split_sync_waits(nc)
    return nc


_NC_CACHE = None


def _get_nc():
    global _NC_CACHE
    if _NC_CACHE is None:
        _NC_CACHE = _build_bass()
    return _NC_CACHE


def kernel(inputs, lut=None, p_q_2_lut_table=None, **_unused):
    x = np.ascontiguousarray(np.asarray(inputs), dtype=np.float32)
    assert x.shape == (B, DEPTH, SIX), x.shape
    shards = x.reshape(N_CORES, P, FD_TOT * SIX)
    in_maps = [{"x": shards[i]} for i in range(N_CORES)]
    res = run_bass_kernel_spmd(_get_nc(), in_maps, list(range(N_CORES)))
    out = np.stack([res.results[i]["out"].reshape(-1) for i in range(N_CORES)])
    return out.reshape(B, DEPTH)

